# revision 11
# baseline (speedup 1.0000x reference)
"""MoE downsample kernel for 8 TRN2 NeuronCores — top-2-only compute.

The reference computes all 4 experts densely but only the host-computable
top-2 gate survives to the output, so the device computes just the 32
selected (sample, expert) convs (seed-0 demand: 1112 taps vs 2624 dense).

Work distribution keeps one compiled SPMD program with perfect balance:
every core computes output rows [16c, 16c+16) of EVERY sample (16
segments/core). Within a segment the input window is column-split across
the two PE row halves (half r covers output cols 64r..64r+63), and the
two PSUM col halves swap (expert, row-block) assignments so each of the
four 64x64 quadrant queues does exactly t_e1 + t_e2 tap-matmuls per
segment. Chunk-task = 8 rows x 64 cols = 512 px = one PSUM bank.
BN + conv-bias + GELU fuse into the ScalarE PSUM eviction. Gating and
final top-2 scale/assembly run on host.

DMA is the co-bottleneck (~47 MB moved against a ~208 GB/s packet-rate
ceiling), so: windows are staged CONTIGUOUSLY by the host and DMAed flat
(10-20 KB packets instead of ~300 B window rows), outputs are stored
bf16 (halves output bytes) on the scalar HWDGE ring while windows ride
the sync HWDGE ring, window loads are emitted two segments ahead, PSUM
banks are shared pairwise across the PSUM col halves for slack, and
light segments go first/last to shorten the DMA-bound head and the
eviction tail.
"""

import numpy as np
import ml_dtypes

KS = [3, 5, 7, 9]
DS = [1, 2, 3, 4]
PADS = [1, 4, 9, 16]       # d*(k-1)//2
TAPN = [9, 25, 49, 81]
BN_EPS = 1e-5
B, CIN, H, W = 16, 64, 256, 256
CE = 64
HO = WO = 128
NCORES = 8
PAD = 16                   # left/top pad in the padded image
HP = WP = PAD + 256 + 15   # 287
NSEG = B                   # one segment per sample per core
RB = 8                     # output rows per chunk-task block
WIN_R = 63                 # max window rows (pair pad 16)
WIN_W = 159                # max window cols per half: 127 + 2*16
WIN_FLAT = WIN_R * WIN_W   # flat per-partition window capacity
PF = 3                     # window prefetch distance (segments)
NWARM = 48                 # dummy matmuls to pre-warm the PE HAM clock

_COMPILED = {}


def _tap_offsets(e):
    """Yield (local_slot, row_off, col_off) raw offsets for expert e."""
    k, d = KS[e], DS[e]
    pad = d * (k - 1) // 2
    for u in range(k):
        for v in range(k):
            yield u * k + v, d * u - pad, d * v - pad


def _seg_order(pairs):
    """Light segments (small windows) lead so the DMA pipeline fills
    fast, one light segment closes the run (short eviction tail), and
    the rest sit light-between-heavies for prefetch headroom."""
    taps = [TAPN[a] + TAPN[b] for a, b in pairs]
    by_weight = sorted(range(NSEG), key=lambda s: (taps[s], s))
    nl = min((NSEG + 2) // 3, NSEG)
    light_pos = sorted({0, 1, 3, 6, 9, NSEG - 1} & set(range(NSEG)))
    light_pos = light_pos[:nl]
    order = [None] * NSEG
    for pos, s in zip(light_pos, by_weight[:len(light_pos)]):
        order[pos] = s
    rest = iter(sorted(by_weight[len(light_pos):],
                       key=lambda s: (-taps[s], s)))
    for i in range(NSEG):
        if order[i] is None:
            order[i] = next(rest)
    return order


def _build_program(pairs, order, compile=True):
    import concourse.bass as bass  # noqa: F401
    import concourse.mybir as mybir
    import concourse.tile as tile
    from concourse import bacc
    from contextlib import ExitStack

    dt = mybir.dt
    nc = bacc.Bacc("TRN2", target_bir_lowering=False, debug=False,
                   num_devices=NCORES)
    xwin = nc.dram_tensor("xwin", [NSEG, 2, CIN, WIN_FLAT], dt.bfloat16,
                          kind="ExternalInput")
    wt = nc.dram_tensor("wt", [CIN, sum(TAPN), CE], dt.bfloat16,
                        kind="ExternalInput")
    bnp = nc.dram_tensor("bnp", [CE, 4, 2], dt.float32, kind="ExternalInput")
    # [seg, rank, row-block, col-half, ch, row, col]
    out = nc.dram_tensor("out", [NSEG, 2, 2, 2, CE, RB, 64], dt.bfloat16,
                         kind="ExternalOutput")
    slot_base = np.cumsum([0] + [k * k for k in KS]).tolist()

    with tile.TileContext(nc) as tc:
        with ExitStack() as ctx:
            consts = ctx.enter_context(tc.tile_pool(name="consts", bufs=1))
            win_pool = ctx.enter_context(tc.tile_pool(name="win", bufs=5))
            stage_pool = ctx.enter_context(tc.tile_pool(name="st", bufs=8))
            psum_pool = ctx.enter_context(
                tc.tile_pool(name="ps", bufs=8, space="PSUM"))

            # per-expert weight tiles so early matmuls only wait on the
            # weights they actually use; first segment's experts load first
            wts = {}
            bntile = consts.tile([128, 4, 2], dt.float32)
            e_order = list(pairs[order[0]])
            e_order += [e for e in range(4) if e not in e_order]

            def load_weights(e):
                t = consts.tile([128, KS[e] * KS[e], CE], dt.bfloat16,
                                tag=f"wt{e}", name=f"wt{e}")
                for half in range(2):
                    p0 = half * 64
                    nc.sync.dma_start(
                        out=t[p0:p0 + 64, :, :],
                        in_=wt[:, slot_base[e]:slot_base[e] + KS[e] * KS[e],
                               :])
                wts[e] = t

            win_tiles = {}

            def issue_window(j):
                seg = order[j]
                e1, e2 = pairs[seg]
                p = max(PADS[e1], PADS[e2])
                R = 31 + 2 * p
                Wd = 127 + 2 * p
                win = win_pool.tile([128, WIN_FLAT], dt.bfloat16, name="win")
                # one col-half per HWDGE ring so the halves transfer
                # concurrently instead of draining FIFO on one ring
                for r, eng in ((0, nc.sync), (1, nc.scalar)):
                    eng.dma_start(
                        out=win[64 * r:64 * r + 64, 0:R * Wd],
                        in_=xwin[seg, r, :, 0:R * Wd])
                win_tiles[j] = win

            load_weights(e_order[0])
            load_weights(e_order[1])
            for half in range(2):
                p0 = half * 64
                nc.sync.dma_start(out=bntile[p0:p0 + 64, :, :],
                                  in_=bnp.ap())
            issue_window(0)
            for e in e_order[2:]:
                load_weights(e)
            for j in range(1, 1 + PF):
                issue_window(j)

            # keep the PE HAM clock warm through the startup DMA wait:
            # zero matmuls into the first segment's bank; the real first
            # tap has start=True so the garbage is overwritten
            scratch = consts.tile([128, 512], dt.bfloat16)
            nc.vector.memset(scratch, 0.0)
            warm_ps = None

            for j in range(NSEG):
                seg = order[j]
                e1, e2 = pairs[seg]
                p = max(PADS[e1], PADS[e2])
                Wd = 127 + 2 * p
                R = 31 + 2 * p
                if j + PF + 1 < NSEG:
                    issue_window(j + PF + 1)
                win = win_tiles.pop(j)
                # [128, R, Wd] strided view of the flat-packed window
                winv = win[:, 0:R * Wd].rearrange("p (r w) -> p r w", w=Wd)
                # PSUM banks shared across the two col-halves per
                # (phase, row-half): 4 live banks per segment
                ps_seg = {(ph, r): psum_pool.tile([128, 512], dt.float32,
                                                  name="psb")
                          for ph in range(2) for r in range(2)}
                if j == 0:
                    warm_ps = ps_seg[(0, 0)]
                    for _ in range(NWARM):
                        nc.tensor.matmul(warm_ps[0:64, :], scratch[0:64, 0:64],
                                         scratch[0:64, :], start=True,
                                         stop=True, tile_position=(0, 0))

                def task_events(e, blk, r, c, ph, p=p, winv=winv, seg=seg,
                                e1=e1, ps_seg=ps_seg):
                    ps = ps_seg[(ph, r)]
                    q0 = c * 64
                    p0 = r * 64
                    taps = list(_tap_offsets(e))
                    wte = wts[e]
                    for t, (slot, ro, co) in enumerate(taps):
                        r_lo = 16 * blk + ro + p
                        c_lo = co + p
                        rhs = winv[p0:p0 + 64, r_lo:r_lo + 15:2,
                                   c_lo:c_lo + 127:2]
                        lhsT = wte[p0:p0 + 64, slot, :]
                        psv = ps[q0:q0 + 64, :]
                        first = t == 0
                        last = t == len(taps) - 1

                        def mm(rhs=rhs, lhsT=lhsT, psv=psv, first=first,
                               last=last, p0=p0, q0=q0):
                            nc.tensor.matmul(psv, lhsT, rhs, start=first,
                                             stop=last,
                                             tile_position=(p0, q0))
                        yield mm

                    def evict(ps=ps, e=e, blk=blk, r=r, q0=q0, seg=seg,
                              e1=e1):
                        st = stage_pool.tile([128, RB, 64], dt.bfloat16,
                                             name="st")
                        nc.scalar.activation(
                            st[q0:q0 + 64, :, :],
                            ps[q0:q0 + 64, :].rearrange(
                                "p (a b) -> p a b", a=RB),
                            mybir.ActivationFunctionType.Gelu,
                            scale=bntile[q0:q0 + 64, e, 0:1],
                            bias=bntile[q0:q0 + 64, e, 1:2])
                        rank = 0 if e == e1 else 1
                        nc.scalar.dma_start(
                            out=out[seg, rank, blk, r, :, :, :],
                            in_=st[q0:q0 + 64, :, :])
                    yield evict

                def qgen(r, c, e1=e1, e2=e2):
                    ea, eb = (e1, e2) if c == 0 else (e2, e1)
                    yield from task_events(ea, 0, r, c, 0)
                    yield from task_events(eb, 1, r, c, 1)

                queues = [qgen(r, c) for r in range(2) for c in range(2)]
                live = list(queues)
                while live:
                    nxt = []
                    for q in live:
                        ev = next(q, None)
                        if ev is None:
                            continue
                        ev()
                        nxt.append(q)
                    live = nxt

    if compile:
        nc.compile()
    return nc


def _get_program(pairs, order):
    key = (pairs, tuple(order))
    if key not in _COMPILED:
        _COMPILED[key] = _build_program(pairs, order)
    return _COMPILED[key]


def _host_gate(x, gate_w, gate_b):
    """Replicate reference gating in numpy (f64 pooling for robustness)."""
    pooled = x.astype(np.float64).mean(axis=(2, 3)).astype(np.float32)
    logits = pooled @ gate_w.T.astype(np.float32) + gate_b
    z = logits - logits.max(axis=1, keepdims=True)
    ez = np.exp(z.astype(np.float32))
    gates = ez / ez.sum(axis=1, keepdims=True)
    idx = np.argsort(-gates, axis=1, kind="stable")[:, :2]
    wsel = np.take_along_axis(gates, idx, axis=1)
    wsel = wsel / (wsel.sum(axis=1, keepdims=True) + 1e-8)
    return idx, wsel.astype(np.float32)


def _prep_inputs(x, ws, bs, bn_scale, bn_bias, bn_mean, bn_var, pairs):
    bf16 = ml_dtypes.bfloat16
    slot_base = np.cumsum([0] + [k * k for k in KS]).tolist()
    xpad = np.zeros((B, CIN, HP, WP), dtype=bf16)
    xpad[:, :, PAD:PAD + H, PAD:PAD + W] = x.astype(bf16)

    # contiguous flat windows per (core, sample, col-half)
    xwin_cores = []
    for c in range(NCORES):
        xw = np.zeros((NSEG, 2, CIN, WIN_FLAT), dtype=bf16)
        for s in range(NSEG):
            e1, e2 = pairs[s]
            p = max(PADS[e1], PADS[e2])
            R = 31 + 2 * p
            Wd = 127 + 2 * p
            r0 = 32 * c + 16 - p
            for r in range(2):
                src0 = (16 - p) + 128 * r
                xw[s, r, :, 0:R * Wd] = (
                    xpad[s, :, r0:r0 + R, src0:src0 + Wd]
                    .reshape(CIN, R * Wd))
        xwin_cores.append(xw)

    wt = np.empty((CIN, sum(TAPN), CE), dtype=bf16)
    for e in range(4):
        k = KS[e]
        w = ws[e].astype(np.float32)  # [CE, CIN, k, k]
        wt[:, slot_base[e]:slot_base[e] + k * k, :] = (
            w.transpose(1, 2, 3, 0).reshape(CIN, k * k, CE).astype(bf16))

    inv = (bn_scale / np.sqrt(bn_var + BN_EPS)).astype(np.float32)
    shift = (np.stack(bs) * inv + bn_bias - bn_mean * inv).astype(np.float32)
    bnp = np.stack([inv, shift], axis=1)  # [4, 2, CE]
    bnp = np.ascontiguousarray(bnp.transpose(2, 0, 1))  # [CE, 4, 2]
    return xwin_cores, wt, bnp


def run(inputs, trace=False):
    from concourse import bass_utils

    x = np.asarray(inputs["x"], dtype=np.float32)
    ws = [np.asarray(inputs[f"w{i}"], dtype=np.float32) for i in range(4)]
    bs = [np.asarray(inputs[f"b{i}"], dtype=np.float32) for i in range(4)]
    bn_scale = np.asarray(inputs["bn_scale"], dtype=np.float32)
    bn_bias = np.asarray(inputs["bn_bias"], dtype=np.float32)
    bn_mean = np.asarray(inputs["bn_mean"], dtype=np.float32)
    bn_var = np.asarray(inputs["bn_var"], dtype=np.float32)
    gate_w = np.asarray(inputs["gate_w"], dtype=np.float32)
    gate_b = np.asarray(inputs["gate_b"], dtype=np.float32)

    idx, wsel = _host_gate(x, gate_w, gate_b)
    pairs = tuple((int(idx[s, 0]), int(idx[s, 1])) for s in range(B))
    order = _seg_order(pairs)

    nc = _get_program(pairs, order)
    xwin_cores, wt, bnp = _prep_inputs(x, ws, bs, bn_scale, bn_bias,
                                       bn_mean, bn_var, pairs)
    in_maps = [{"xwin": xwin_cores[c], "wt": wt, "bnp": bnp}
               for c in range(NCORES)]
    res = bass_utils.run_bass_kernel_spmd(
        nc, in_maps, core_ids=list(range(NCORES)), trace=trace)

    # assemble: core c holds rows [16c, 16c+16) of every (sample, rank)
    E = np.empty((B, 2, CE, HO, WO), dtype=np.float32)
    for c in range(NCORES):
        o = res.results[c]["out"]  # [seg, rank, blk, rhalf, ch, row, col]
        t = (o.astype(np.float32)
             .transpose(0, 1, 4, 2, 5, 3, 6).reshape(NSEG, 2, CE, 16, 128))
        E[:, :, :, 16 * c:16 * c + 16, :] = t
    outf = np.empty((B, 2 * CE, HO, WO), dtype=np.float32)
    for s in range(B):
        outf[s, :CE] = E[s, 0] * wsel[s, 0]
        outf[s, CE:] = E[s, 1] * wsel[s, 1]
    return outf, res


def kernel(**inputs):
    outf, _ = run(inputs, trace=False)
    return outf
